# revision 6
# baseline (speedup 1.0000x reference)
"""Trainium2 Bass kernel for nn_GatedJunction (gated multi-branch junction).

Math (per batch element b):
    m_y  = mean_hw(y[b])                     # [C]
    m_xk = mean_hw(x_k[b])                   # [C] for k=0..3
    feats = concat(m_y, m_x0..m_x3)          # [5C] = [1280]
    h  = relu(bn(feats @ conv1_w.T))         # [32]
    w  = h @ conv2_w.T + conv2_b             # [1280] -> [5, 256]
    w1 = sigmoid(w[0])                       # self gate  [256]
    w2 = softmax_k(w[1:])                    # branch gates [4, 256]
    out[b] = y[b]*w1[:,None,None] + sum_k w2[k][:,None,None]*x_k[b]

Sharding: data-parallel over batch. 8 cores x 4 batch elements each.
Params are tiny, folded on the host (BN affine, weight transposes) and
replicated to every core.

This version is fp16 end-to-end on the HBM side: inputs are cast to
fp16 on the host (halves the DMA traffic of this memory-bound problem)
and the output is stored fp16 and widened on the host. Per-core traffic
drops 24 MiB -> 12 MiB. fp16 keeps ~5e-4 relative precision, far inside
the 2e-2 gate (verified against the fp32 reference: ~6e-4).

On-core layout: channel-on-partition. Each (tensor, batch) is one SBUF
tile [128, CH=2, HW=1024] fp16 (4 KiB/partition, contiguous in DRAM per
partition -> 4 KiB DMA descriptors). Work split:
  - DVE:   per-channel sums via tensor_scalar(accum_out=) (fp16 4x mode),
           softmax bits, half the PSUM->SBUF copies.
  - PE:    the gate MLP (fp32) and ALL of pass 2: out_chunk = sum_t
           diag(gate_t) @ x_t_chunk as 5-matmul PSUM accumulation groups
           with diagonal fp16 weights (PE is otherwise idle).
  - GpSimd: builds the 10 diagonal weight tiles (identity * gate column,
           with the softmax normalization folded in as a second scalar).
  - ACT:   BN+ReLU, sigmoid/exp, the other half of the PSUM->SBUF copies.
  - Stores go on the ACT HWDGE ring so the SP ring carries only loads.
"""

import sys

for _p in ("/root/.axon_site/_ro/trn_rl_repo", "/opt/trn_rl_repo"):
    if _p not in sys.path:
        sys.path.append(_p)

from contextlib import ExitStack

import numpy as np

import concourse.bass as bass
import concourse.tile as tile
from concourse import mybir
from concourse.bass_utils import run_bass_kernel_spmd

# Problem constants (hardcoded from the spec).
B, K, C, H, W = 32, 4, 256, 32, 32
MID = 32
EPS = 1e-5
HW = H * W          # 1024
N_CORES = 8
B_LOC = B // N_CORES  # 4
NT = K + 1          # 5 tensors: y, x0..x3
FEAT = NT * C       # 1280
NCH = FEAT // 128   # 10 feature chunks of 128
CH = C // 128       # 2 channel chunks per tensor
HH = HW // 2        # 512 = one PSUM bank of fp32

FP32 = mybir.dt.float32
FP16 = mybir.dt.float16
ALU = mybir.AluOpType
AF = mybir.ActivationFunctionType


def _split_waits(nc: bass.Bass) -> None:
    """This toolchain's walrus accepts only ONE sync-wait per instruction
    (setupSyncWait: 'Too many sync wait commands') while Tile emits several.
    Hoist all-but-one wait onto standalone EventSemaphore instructions
    placed immediately before, on the same engine — semantically identical
    (sequencer stalls at each wait in order)."""
    for f in nc.m.functions:
        for blk in f.blocks:
            insts = list(blk.instructions)
            out, changed = [], False
            for inst in insts:
                si = inst.sync_info
                if si is not None and len(si.on_wait) > 1:
                    waits = list(si.on_wait)
                    for i, w in enumerate(waits[:-1]):
                        ev = mybir.InstEventSemaphore(
                            name=f"{inst.name}-sw{i}", ins=[], outs=[]
                        )
                        ev.engine = inst.engine
                        ev.sync_info = mybir.SyncInfo(on_wait=[w], on_update=[])
                        out.append(ev)
                    si.on_wait = [waits[-1]]
                    changed = True
                out.append(inst)
            if changed:
                blk.instructions = out


def build_program(debug: bool = False, repeat: int = 1) -> bass.Bass:
    """Emit the single-core SPMD program (same program, per-core data).

    repeat > 1 re-runs the whole batch loop (idempotent) — used only for
    launch-overhead-cancelling timing in test.py.
    """
    nc = bass.Bass()
    if debug:
        d_dbg_sums = nc.declare_dram_parameter("dbg_sums", [B_LOC, 128, NCH], FP32, isOutput=True)
        d_dbg_h = nc.declare_dram_parameter("dbg_h", [B_LOC, MID, 1], FP32, isOutput=True)
        d_dbg_lg = nc.declare_dram_parameter("dbg_lg", [B_LOC, 128, NCH], FP32, isOutput=True)
        d_dbg_gs = nc.declare_dram_parameter("dbg_gs", [B_LOC, 128, CH], FP32, isOutput=True)
        d_dbg_gn = nc.declare_dram_parameter("dbg_gn", [B_LOC, 128, K, CH], FP32, isOutput=True)
        d_dbg_dg = nc.declare_dram_parameter("dbg_dg", [B_LOC, 128, 128], FP16, isOutput=True)

    d_in = [
        nc.declare_dram_parameter(nm, [B_LOC, 128, CH, HW], FP16, isOutput=False)
        for nm in ("y", "x0", "x1", "x2", "x3")
    ]
    # Pre-transposed / pre-folded params (host side):
    #   w1T[p, j, m] = conv1_w[m, 128j+p]
    #   w2T[m, j, p] = conv2_w[128j+p, m]
    #   c2bT[p, j]   = conv2_b[128j+p]
    #   scale_eff = (gamma/sqrt(var+eps))/HW, bias_eff = beta - mean*gamma/sqrt(var+eps)
    d_w1T = nc.declare_dram_parameter("w1T", [128, NCH, MID], FP32, isOutput=False)
    d_w2T = nc.declare_dram_parameter("w2T", [MID, NCH, 128], FP32, isOutput=False)
    d_c2bT = nc.declare_dram_parameter("c2bT", [128, NCH], FP32, isOutput=False)
    d_scale = nc.declare_dram_parameter("scale_eff", [MID, 1], FP32, isOutput=False)
    d_bias = nc.declare_dram_parameter("bias_eff", [MID, 1], FP32, isOutput=False)
    d_ident = nc.declare_dram_parameter("ident", [128, 128], FP16, isOutput=False)
    d_out = nc.declare_dram_parameter("out", [B_LOC, 128, CH, HW], FP16, isOutput=True)

    with tile.TileContext(nc) as tc, ExitStack() as ctx:
        cpool = ctx.enter_context(tc.tile_pool(name="cpool", bufs=1))
        ppool = ctx.enter_context(tc.tile_pool(name="ppool", bufs=1, space="PSUM"))
        dpool = ctx.enter_context(tc.tile_pool(name="dpool", bufs=2))
        spool = ctx.enter_context(tc.tile_pool(name="spool", bufs=2))

        # ---------------- parameter loads (once) ----------------
        w1T = cpool.tile([128, NCH, MID], FP32, name="w1T", tag="w1T")
        nc.sync.dma_start(out=w1T[:], in_=d_w1T[:])
        w2T = cpool.tile([MID, NCH, 128], FP32, name="w2T", tag="w2T")
        nc.sync.dma_start(out=w2T[:], in_=d_w2T[:])
        c2bT = cpool.tile([128, NCH], FP32, name="c2bT", tag="c2bT")
        nc.sync.dma_start(out=c2bT[:], in_=d_c2bT[:])
        scale_eff = cpool.tile([MID, 1], FP32, name="scale_eff", tag="scale_eff")
        nc.sync.dma_start(out=scale_eff[:], in_=d_scale[:])
        bias_eff = cpool.tile([MID, 1], FP32, name="bias_eff", tag="bias_eff")
        nc.sync.dma_start(out=bias_eff[:], in_=d_bias[:])
        ident = cpool.tile([128, 128], FP16, name="ident", tag="ident")
        nc.sync.dma_start(out=ident[:], in_=d_ident[:])

        # ---------------- main loop over local batches ----------------
        for b in [i % B_LOC for i in range(B_LOC * repeat)]:
            # Load the 5 feature maps for this batch: [128, ch, hw] fp16.
            tiles = []
            for t in range(NT):
                dt_ = dpool.tile(
                    [128, CH, HW], FP16, name=f"d{t}", tag=f"d{t}", bufs=4
                )
                nc.sync.dma_start(out=dt_[:], in_=d_in[t][b])
                tiles.append(dt_)

            # Channel sums -> sums_t[:, j], j = t*CH + ch (fp32 accum).
            sums_t = spool.tile([128, NCH], FP32, name="sums_t", tag="sums_t", bufs=2)
            for t in range(NT):
                for ch in range(CH):
                    j = t * CH + ch
                    scr_v = spool.tile(
                        [128, HW], FP16, name="scr_v", tag="scr_v", bufs=2
                    )
                    nc.vector.tensor_scalar(
                        out=scr_v[:],
                        in0=tiles[t][:, ch, :],
                        scalar1=1.0,
                        scalar2=None,
                        op0=ALU.mult,
                        op1=ALU.add,
                        accum_out=sums_t[:, j : j + 1],
                    )

            # Gate MLP on PE (fp32): h_raw[mid] = sum_j w1T[:,j,:].T @ sums[:,j]
            hps = ppool.tile([MID, 1], FP32, name="hps", tag="hps", bufs=2)
            for j in range(NCH):
                nc.tensor.matmul(
                    hps[:],
                    w1T[:, j, :],
                    sums_t[:, j : j + 1],
                    start=(j == 0),
                    stop=(j == NCH - 1),
                )
            h_sb = spool.tile([MID, 1], FP32, name="h_sb", tag="h_sb", bufs=2)
            nc.scalar.activation(
                out=h_sb[:], in_=hps[:], func=AF.Relu,
                bias=bias_eff[:], scale=scale_eff[:],
            )
            # Logits (pre-bias), transposed into channel-on-partition layout:
            # wps[p, j] = w[128j + p] - conv2_b[128j + p]
            wps = ppool.tile([128, NCH], FP32, name="wps", tag="wps", bufs=2)
            for j in range(NCH):
                nc.tensor.matmul(
                    wps[:, j : j + 1], w2T[:, j, :], h_sb[:], start=True, stop=True
                )
            # Add conv2 bias on DVE (per-column biases; ACT bias is per-partition).
            lg = spool.tile([128, NCH], FP32, name="lg", tag="lg", bufs=2)
            nc.vector.tensor_tensor(out=lg[:], in0=wps[:], in1=c2bT[:], op=ALU.add)

            # Gates: cols 0..CH-1 sigmoid self gate; cols CH.. exp for softmax.
            gat_s = spool.tile([128, CH], FP32, name="gat_s", tag="gat_s", bufs=2)
            nc.scalar.activation(out=gat_s[:], in_=lg[:, 0:CH], func=AF.Sigmoid)
            gat_e = spool.tile([128, K * CH], FP32, name="gat_e", tag="gat_e", bufs=2)
            nc.scalar.activation(out=gat_e[:], in_=lg[:, CH:NCH], func=AF.Exp)
            # softmax denominators over k: gat_e cols are (k, ch) k-major.
            gk = gat_e[:].rearrange("p (k c) -> p c k", c=CH)
            esum = spool.tile([128, CH, 1], FP32, name="esum", tag="esum", bufs=2)
            nc.vector.reduce_sum(out=esum[:], in_=gk, axis=mybir.AxisListType.X)
            rinv = spool.tile([128, CH, 1], FP32, name="rinv", tag="rinv", bufs=2)
            nc.vector.reciprocal(rinv[:], esum[:])

            # Normalized branch gates: gat_n[:, k, ch] = gat_e[:, k*CH+ch] * rinv[ch].
            gat_n = spool.tile([128, K, CH], FP32, name="gat_n", tag="gat_n", bufs=2)
            ge_v = gat_e[:].rearrange("p (k c) -> p k c", c=CH)
            for ch in range(CH):
                nc.vector.tensor_scalar_mul(
                    out=gat_n[:, :, ch], in0=ge_v[:, :, ch], scalar1=rinv[:, ch, :]
                )

            # Diagonal gate-weight tiles for PE (fp16): ident * gate column.
            # Built on ACT (activation Copy with per-partition scale).
            diags = []  # diags[t][ch]
            for t in range(NT):
                dg_t = []
                for ch in range(CH):
                    col = (
                        gat_s[:, ch : ch + 1]
                        if t == 0
                        else gat_n[:, t - 1, ch : ch + 1]
                    )
                    dg = spool.tile(
                        [128, 128], FP16, name=f"dg{t}{ch}", tag=f"dg{t}{ch}", bufs=2
                    )
                    nc.scalar.activation(
                        out=dg[:], in_=ident[:], func=AF.Copy, scale=col
                    )
                    dg_t.append(dg)
                diags.append(dg_t)

            if debug:
                nc.sync.dma_start(out=d_dbg_sums[b], in_=sums_t[:])
                nc.sync.dma_start(out=d_dbg_h[b], in_=h_sb[:])
                nc.sync.dma_start(out=d_dbg_lg[b], in_=lg[:])
                nc.sync.dma_start(out=d_dbg_gs[b], in_=gat_s[:])
                nc.sync.dma_start(out=d_dbg_gn[b], in_=gat_n[:])
                nc.sync.dma_start(out=d_dbg_dg[b], in_=diags[1][0][:])

            # Pass 2 on PE: acc = diag(w1)@y + sum_k diag(w2k)@x_k, per
            # (ch, half) as a 5-matmul accumulation group into one PSUM bank.
            outsb = dpool.tile([128, CH, HW], FP16, name="outsb", tag="outsb", bufs=2)
            for ch in range(CH):
                for h in range(2):
                    fs = slice(h * HH, (h + 1) * HH)
                    ps = ppool.tile([128, HH], FP32, name="ps", tag="ps", bufs=4)
                    for t in range(NT):
                        nc.tensor.matmul(
                            ps[:],
                            diags[t][ch][:],
                            tiles[t][:, ch, fs],
                            start=(t == 0),
                            stop=(t == NT - 1),
                        )
                    # PSUM -> SBUF (fp16), split DVE / ACT.
                    if h == 0:
                        nc.vector.tensor_copy(outsb[:, ch, fs], ps[:])
                    else:
                        nc.scalar.activation(
                            out=outsb[:, ch, fs], in_=ps[:], func=AF.Copy
                        )
            # Store on the ACT HWDGE ring (keeps SP ring load-only).
            nc.scalar.dma_start(out=d_out[b], in_=outsb[:])

    _split_waits(nc)
    return nc


_CACHE: dict = {}


def _get_program() -> bass.Bass:
    if "nc" not in _CACHE:
        _CACHE["nc"] = build_program()
    return _CACHE["nc"]


def make_in_maps(inputs: dict) -> list:
    """Shard full inputs into per-core input maps (batch-parallel)."""
    f32 = lambda a: np.asarray(a, dtype=np.float32)
    # [B, C, H, W] -> [B, 128, CH, HW] fp16, channel-on-partition (c = ch*128+p).
    def prep(a):
        a = f32(a).reshape(B, CH, 128, HW).transpose(0, 2, 1, 3)
        return np.ascontiguousarray(a, dtype=np.float16)

    y = prep(inputs["y"])
    xs = [prep(inputs[f"x{k}"]) for k in range(K)]

    conv1_w = f32(inputs["conv1_w"])
    conv2_w = f32(inputs["conv2_w"])
    gamma = f32(inputs["bn_gamma"])
    beta = f32(inputs["bn_beta"])
    mean = f32(inputs["bn_mean"])
    var = f32(inputs["bn_var"])
    s = gamma / np.sqrt(var + EPS)
    shared = {
        "w1T": np.ascontiguousarray(
            conv1_w.reshape(MID, NCH, 128).transpose(2, 1, 0)
        ),
        "w2T": np.ascontiguousarray(
            conv2_w.reshape(NCH, 128, MID).transpose(2, 0, 1)
        ),
        "c2bT": np.ascontiguousarray(f32(inputs["conv2_b"]).reshape(NCH, 128).T),
        "scale_eff": np.ascontiguousarray((s / HW).reshape(MID, 1)),
        "bias_eff": np.ascontiguousarray((beta - mean * s).reshape(MID, 1)),
        "ident": np.eye(128, dtype=np.float16),
    }
    in_maps = []
    for core in range(N_CORES):
        sl = slice(core * B_LOC, (core + 1) * B_LOC)
        m = {"y": np.ascontiguousarray(y[sl])}
        for k in range(K):
            m[f"x{k}"] = np.ascontiguousarray(xs[k][sl])
        m.update(shared)
        in_maps.append(m)
    return in_maps


def kernel(**inputs) -> np.ndarray:
    nc = _get_program()
    in_maps = make_in_maps(inputs)
    res = run_bass_kernel_spmd(nc, in_maps, list(range(N_CORES)))
    _CACHE["last_results"] = res
    # out [B_LOC, 128, CH, HW] fp16 -> [B_LOC, C, H, W] f32 (c = ch*128+p).
    out = np.concatenate(
        [
            np.asarray(res.results[i]["out"], dtype=np.float32)
            .transpose(0, 2, 1, 3)
            .reshape(B_LOC, C, H, W)
            for i in range(N_CORES)
        ],
        axis=0,
    )
    return out


# revision 7
# speedup vs baseline: 1.3277x; 1.3277x over previous
"""Trainium2 Bass kernel for nn_GatedJunction (gated multi-branch junction).

Math (per batch element b):
    m_y  = mean_hw(y[b])                     # [C]
    m_xk = mean_hw(x_k[b])                   # [C] for k=0..3
    feats = concat(m_y, m_x0..m_x3)          # [5C] = [1280]
    h  = relu(bn(feats @ conv1_w.T))         # [32]
    w  = h @ conv2_w.T + conv2_b             # [1280] -> [5, 256]
    w1 = sigmoid(w[0])                       # self gate  [256]
    w2 = softmax_k(w[1:])                    # branch gates [4, 256]
    out[b] = y[b]*w1[:,None,None] + sum_k w2[k][:,None,None]*x_k[b]

Sharding: data-parallel over batch. 8 cores x 4 batch elements each.
Params are tiny, folded on the host (BN affine, weight transposes) and
replicated to every core.

fp16 end-to-end on the HBM side: inputs are cast to fp16 on the host
(halves the DMA traffic of this memory-bound problem; per-core traffic
24 MiB -> 12 MiB) and the output is stored fp16 and widened on the
host. fp16 keeps ~6e-4 relative error, far inside the 2e-2 gate.

On-core layout: channel-on-partition; batches are loaded in PAIRS (one
1 MiB DMA per tensor per pair -> 10 loads + 2 stores per 4-batch pass;
fewer, larger DMAs measured faster than per-batch 512 KiB ones).
Engine split (HW evidence: PE sequencer dispatch is expensive, so PE
only runs the tiny gate MLP; bulk elementwise is DVE with ACT overflow):
  - DVE:  channel sums for y/x0/x1 (tensor_scalar accum, fp16 4x mode),
          softmax bits, and the 4-step scalar_tensor_tensor chains
          acc += w2k * xk for both channel halves.
  - ACT:  channel sums for x2/x3 (activation accum), BN+ReLU,
          sigmoid/exp, and the chain starts acc = y*w1 (Copy w/ scale).
  - PE:   the 1280->32->1280 gate MLP in fp32 (20 small matmuls).
  - Stores ride the ACT HWDGE ring so the SP ring carries only loads.
"""

import sys

for _p in ("/root/.axon_site/_ro/trn_rl_repo", "/opt/trn_rl_repo"):
    if _p not in sys.path:
        sys.path.append(_p)

from contextlib import ExitStack

import numpy as np

import concourse.bass as bass
import concourse.tile as tile
from concourse import mybir
from concourse.bass_utils import run_bass_kernel_spmd

# Problem constants (hardcoded from the spec).
B, K, C, H, W = 32, 4, 256, 32, 32
MID = 32
EPS = 1e-5
HW = H * W          # 1024
N_CORES = 8
B_LOC = B // N_CORES  # 4
NP = B_LOC // 2     # batch pairs per core
NT = K + 1          # 5 tensors: y, x0..x3
FEAT = NT * C       # 1280
NCH = FEAT // 128   # 10 feature chunks of 128
CH = C // 128       # 2 channel chunks per tensor

FP32 = mybir.dt.float32
FP16 = mybir.dt.float16
ALU = mybir.AluOpType
AF = mybir.ActivationFunctionType


def _split_waits(nc: bass.Bass) -> None:
    """This toolchain's walrus accepts only ONE sync-wait per instruction
    (setupSyncWait: 'Too many sync wait commands') while Tile emits several.
    Hoist all-but-one wait onto standalone EventSemaphore instructions
    placed immediately before, on the same engine — semantically identical
    (sequencer stalls at each wait in order)."""
    for f in nc.m.functions:
        for blk in f.blocks:
            insts = list(blk.instructions)
            out, changed = [], False
            for inst in insts:
                si = inst.sync_info
                if si is not None and len(si.on_wait) > 1:
                    waits = list(si.on_wait)
                    for i, w in enumerate(waits[:-1]):
                        ev = mybir.InstEventSemaphore(
                            name=f"{inst.name}-sw{i}", ins=[], outs=[]
                        )
                        ev.engine = inst.engine
                        ev.sync_info = mybir.SyncInfo(on_wait=[w], on_update=[])
                        out.append(ev)
                    si.on_wait = [waits[-1]]
                    changed = True
                out.append(inst)
            if changed:
                blk.instructions = out


def build_program(debug: bool = False, repeat: int = 1) -> bass.Bass:
    """Emit the single-core SPMD program (same program, per-core data).

    repeat > 1 re-runs the whole batch loop (idempotent) — used only for
    launch-overhead-cancelling timing in test.py.
    """
    nc = bass.Bass()

    d_in = [
        nc.declare_dram_parameter(nm, [B_LOC, 128, CH, HW], FP16, isOutput=False)
        for nm in ("y", "x0", "x1", "x2", "x3")
    ]
    # Pre-transposed / pre-folded params (host side):
    #   w1T[p, j, m] = conv1_w[m, 128j+p]
    #   w2T[m, j, p] = conv2_w[128j+p, m]
    #   c2bT[p, j]   = conv2_b[128j+p]
    #   scale_eff = (gamma/sqrt(var+eps))/HW, bias_eff = beta - mean*gamma/sqrt(var+eps)
    d_w1T = nc.declare_dram_parameter("w1T", [128, NCH, MID], FP32, isOutput=False)
    d_w2T = nc.declare_dram_parameter("w2T", [MID, NCH, 128], FP32, isOutput=False)
    d_c2bT = nc.declare_dram_parameter("c2bT", [128, NCH], FP32, isOutput=False)
    d_scale = nc.declare_dram_parameter("scale_eff", [MID, 1], FP32, isOutput=False)
    d_bias = nc.declare_dram_parameter("bias_eff", [MID, 1], FP32, isOutput=False)
    d_out = nc.declare_dram_parameter("out", [B_LOC, 128, CH, HW], FP16, isOutput=True)

    with tile.TileContext(nc) as tc, ExitStack() as ctx:
        cpool = ctx.enter_context(tc.tile_pool(name="cpool", bufs=1))
        ppool = ctx.enter_context(tc.tile_pool(name="ppool", bufs=1, space="PSUM"))
        dpool = ctx.enter_context(tc.tile_pool(name="dpool", bufs=2))
        spool = ctx.enter_context(tc.tile_pool(name="spool", bufs=2))

        # ---------------- parameter loads (once) ----------------
        w1T = cpool.tile([128, NCH, MID], FP32, name="w1T", tag="w1T")
        nc.sync.dma_start(out=w1T[:], in_=d_w1T[:])
        w2T = cpool.tile([MID, NCH, 128], FP32, name="w2T", tag="w2T")
        nc.sync.dma_start(out=w2T[:], in_=d_w2T[:])
        c2bT = cpool.tile([128, NCH], FP32, name="c2bT", tag="c2bT")
        nc.sync.dma_start(out=c2bT[:], in_=d_c2bT[:])
        scale_eff = cpool.tile([MID, 1], FP32, name="scale_eff", tag="scale_eff")
        nc.sync.dma_start(out=scale_eff[:], in_=d_scale[:])
        bias_eff = cpool.tile([MID, 1], FP32, name="bias_eff", tag="bias_eff")
        nc.sync.dma_start(out=bias_eff[:], in_=d_bias[:])

        # ---------------- main loop over local batch pairs ----------------
        for p in [i % NP for i in range(NP * repeat)]:
            # One 1 MiB DMA per tensor for the batch pair: [128, 2, CH, HW].
            tiles2 = []
            for t in range(NT):
                dt_ = dpool.tile(
                    [128, 2, CH, HW], FP16, name=f"d{t}", tag=f"d{t}", bufs=3
                )
                nc.sync.dma_start(
                    out=dt_[:],
                    in_=d_in[t][2 * p : 2 * p + 2].rearrange("b p c f -> p b c f"),
                )
                tiles2.append(dt_)
            acc2 = dpool.tile([128, 2, CH, HW], FP16, name="acc2", tag="acc2", bufs=2)

            for bb in range(2):
                # Channel sums -> sums_t[:, j], j = t*CH + ch (fp32 accum).
                sums_t = spool.tile(
                    [128, NCH], FP32, name="sums_t", tag="sums_t", bufs=2
                )
                for t in range(NT):
                    for ch in range(CH):
                        j = t * CH + ch
                        if t < 3:  # y, x0, x1 on DVE
                            scr_v = spool.tile(
                                [128, HW], FP16, name="scr_v", tag="scr_v", bufs=2
                            )
                            nc.vector.tensor_scalar(
                                out=scr_v[:],
                                in0=tiles2[t][:, bb, ch, :],
                                scalar1=1.0,
                                scalar2=None,
                                op0=ALU.mult,
                                op1=ALU.add,
                                accum_out=sums_t[:, j : j + 1],
                            )
                        else:  # x2, x3 on ACT
                            scr_a = spool.tile(
                                [128, HW], FP16, name="scr_a", tag="scr_a", bufs=2
                            )
                            nc.scalar.activation(
                                out=scr_a[:],
                                in_=tiles2[t][:, bb, ch, :],
                                func=AF.Copy,
                                accum_out=sums_t[:, j : j + 1],
                            )

                # Gate MLP on PE (fp32): h_raw = sum_j w1T[:,j,:].T @ sums[:,j]
                hps = ppool.tile([MID, 1], FP32, name="hps", tag="hps", bufs=2)
                for j in range(NCH):
                    nc.tensor.matmul(
                        hps[:],
                        w1T[:, j, :],
                        sums_t[:, j : j + 1],
                        start=(j == 0),
                        stop=(j == NCH - 1),
                    )
                h_sb = spool.tile([MID, 1], FP32, name="h_sb", tag="h_sb", bufs=2)
                nc.scalar.activation(
                    out=h_sb[:], in_=hps[:], func=AF.Relu,
                    bias=bias_eff[:], scale=scale_eff[:],
                )
                # Logits (pre-bias) in channel-on-partition layout.
                wps = ppool.tile([128, NCH], FP32, name="wps", tag="wps", bufs=2)
                for j in range(NCH):
                    nc.tensor.matmul(
                        wps[:, j : j + 1], w2T[:, j, :], h_sb[:], start=True, stop=True
                    )
                # Add conv2 bias (per-column, so DVE not ACT).
                lg = spool.tile([128, NCH], FP32, name="lg", tag="lg", bufs=2)
                nc.vector.tensor_tensor(out=lg[:], in0=wps[:], in1=c2bT[:], op=ALU.add)

                # Gates.
                gat_s = spool.tile([128, CH], FP32, name="gat_s", tag="gat_s", bufs=2)
                nc.scalar.activation(out=gat_s[:], in_=lg[:, 0:CH], func=AF.Sigmoid)
                gat_e = spool.tile(
                    [128, K * CH], FP32, name="gat_e", tag="gat_e", bufs=2
                )
                nc.scalar.activation(out=gat_e[:], in_=lg[:, CH:NCH], func=AF.Exp)
                gk = gat_e[:].rearrange("p (k c) -> p c k", c=CH)
                esum = spool.tile([128, CH, 1], FP32, name="esum", tag="esum", bufs=2)
                nc.vector.reduce_sum(out=esum[:], in_=gk, axis=mybir.AxisListType.X)
                rinv = spool.tile([128, CH, 1], FP32, name="rinv", tag="rinv", bufs=2)
                nc.vector.reciprocal(rinv[:], esum[:])
                # Normalized branch gates gat_n[:, k, ch].
                gat_n = spool.tile(
                    [128, K, CH], FP32, name="gat_n", tag="gat_n", bufs=2
                )
                ge_v = gat_e[:].rearrange("p (k c) -> p k c", c=CH)
                for ch in range(CH):
                    nc.vector.tensor_scalar_mul(
                        out=gat_n[:, :, ch], in0=ge_v[:, :, ch], scalar1=rinv[:, ch, :]
                    )

                # Pass 2: acc = y*w1 (ACT start), then 4 chained FMAs on DVE.
                for ch in range(CH):
                    accv = acc2[:, bb, ch, :]
                    nc.scalar.activation(
                        out=accv,
                        in_=tiles2[0][:, bb, ch, :],
                        func=AF.Copy,
                        scale=gat_s[:, ch : ch + 1],
                    )
                    for k in range(K):
                        nc.vector.scalar_tensor_tensor(
                            out=accv,
                            in0=tiles2[1 + k][:, bb, ch, :],
                            scalar=gat_n[:, k, ch : ch + 1],
                            in1=accv,
                            op0=ALU.mult,
                            op1=ALU.add,
                        )
            # Pair store on the ACT HWDGE ring (keeps SP ring load-only).
            nc.scalar.dma_start(
                out=d_out[2 * p : 2 * p + 2].rearrange("b p c f -> p b c f"),
                in_=acc2[:],
            )

    _split_waits(nc)
    return nc


_CACHE: dict = {}


def _get_program() -> bass.Bass:
    if "nc" not in _CACHE:
        _CACHE["nc"] = build_program()
    return _CACHE["nc"]


def make_in_maps(inputs: dict) -> list:
    """Shard full inputs into per-core input maps (batch-parallel)."""
    f32 = lambda a: np.asarray(a, dtype=np.float32)
    # [B, C, H, W] -> [B, 128, CH, HW] fp16, channel-on-partition (c = ch*128+p).
    def prep(a):
        a = f32(a).reshape(B, CH, 128, HW).transpose(0, 2, 1, 3)
        return np.ascontiguousarray(a, dtype=np.float16)

    y = prep(inputs["y"])
    xs = [prep(inputs[f"x{k}"]) for k in range(K)]

    conv1_w = f32(inputs["conv1_w"])
    conv2_w = f32(inputs["conv2_w"])
    gamma = f32(inputs["bn_gamma"])
    beta = f32(inputs["bn_beta"])
    mean = f32(inputs["bn_mean"])
    var = f32(inputs["bn_var"])
    s = gamma / np.sqrt(var + EPS)
    shared = {
        "w1T": np.ascontiguousarray(
            conv1_w.reshape(MID, NCH, 128).transpose(2, 1, 0)
        ),
        "w2T": np.ascontiguousarray(
            conv2_w.reshape(NCH, 128, MID).transpose(2, 0, 1)
        ),
        "c2bT": np.ascontiguousarray(f32(inputs["conv2_b"]).reshape(NCH, 128).T),
        "scale_eff": np.ascontiguousarray((s / HW).reshape(MID, 1)),
        "bias_eff": np.ascontiguousarray((beta - mean * s).reshape(MID, 1)),
    }
    in_maps = []
    for core in range(N_CORES):
        sl = slice(core * B_LOC, (core + 1) * B_LOC)
        m = {"y": np.ascontiguousarray(y[sl])}
        for k in range(K):
            m[f"x{k}"] = np.ascontiguousarray(xs[k][sl])
        m.update(shared)
        in_maps.append(m)
    return in_maps


def kernel(**inputs) -> np.ndarray:
    nc = _get_program()
    in_maps = make_in_maps(inputs)
    res = run_bass_kernel_spmd(nc, in_maps, list(range(N_CORES)))
    _CACHE["last_results"] = res
    # out [B_LOC, 128, CH, HW] fp16 -> [B_LOC, C, H, W] f32 (c = ch*128+p).
    out = np.concatenate(
        [
            np.asarray(res.results[i]["out"], dtype=np.float32)
            .transpose(0, 2, 1, 3)
            .reshape(B_LOC, C, H, W)
            for i in range(N_CORES)
        ],
        axis=0,
    )
    return out
